# revision 59
# baseline (speedup 1.0000x reference)
"""CrossAttentionBlock kernel for 8 Trainium2 NeuronCores.

Reference computation (per batch b):
    q = x @ Wq;  k,v = y @ Wkv;  per head: softmax(q k^T / sqrt(dk)) v;
    out = concat_heads @ Wproj + bproj

Sharding: 8 cores = 2 batches x 4 head-groups (4 heads each). Each core
computes the partial output contribution of its 4 heads for its batch;
the host sums the 4 partials per batch and adds the bias.

Design notes (cost-model driven):
  - All matmul operands fp16 (1 cycle/row on PE at any size), PSUM fp32.
  - Attention AV is computed in [i, d] orientation (lhsT = P tile), which
    costs 65 rows per j-chunk instead of 512 -> half the PE rows of the
    S^T orientation, and makes the softmax normalization a per-partition
    scalar multiply (no DRAM broadcast bounce). Rowsums come free as a
    65th "ones" column of V.
  - O^T for the output projection comes from PE transposes (fp16, 128
    rows per [128,64] block) collected in a PSUM fp16 tile, then one DVE
    copy per i-block.
  - The PE clock ramps (0.65 -> 1.2 -> 2.4 GHz) and resets on idle, so the
    schedule keeps PE continuously busy: a flat software pipeline over
    (pair, i-block, head, j-group) runs scores 2 slots ahead of the AV
    consumer of the exp output; projection work (Q/K/V of the other pair,
    transposes, output projection) is interleaved as filler units paced
    by PE-row accounting against the Act engine's exp rate.
"""

import numpy as np

import concourse.bass as bass
import concourse.tile as tile
from concourse import bacc, mybir
from concourse.bass_utils import run_bass_kernel_spmd
from concourse.masks import make_identity

B, LQ, LKV = 2, 2048, 2048
C, CTX, H, DK = 1024, 768, 16, 64
SCALE = DK ** (-0.5)

F32 = mybir.dt.float32
F16 = mybir.dt.float16

NCC = C // 128       # 8   contraction chunks for Q proj
NCTX = CTX // 128    # 6   contraction chunks for K/V proj
NIT = LQ // 512      # 4   i blocks
NJT = LKV // 128     # 16  j chunks
GROUPS = [(g0, 2) for g0 in range(0, NJT, 2)]   # 8 groups of 2 j-chunks

# filler pacing: target PE rows of filler per pipeline slot (the gap
# between the Act engine's exp time per slot and the attention PE work)
FILL_ROWS_PER_SLOT = 880


def build_kernel():
    nc = bacc.Bacc("TRN2", target_bir_lowering=False, debug=False)

    xT = nc.dram_tensor("xT", [C, LQ], F16, kind="ExternalInput").ap()
    yT = nc.dram_tensor("yT", [CTX, LKV], F16, kind="ExternalInput").ap()
    wq = nc.dram_tensor("wq", [C, 256], F16, kind="ExternalInput").ap()
    wk = nc.dram_tensor("wk", [CTX, 256], F16, kind="ExternalInput").ap()
    wv = nc.dram_tensor("wv", [CTX, 256], F16, kind="ExternalInput").ap()
    wp = nc.dram_tensor("wp", [256, C], F16, kind="ExternalInput").ap()
    outT = nc.dram_tensor("outT", [C, LQ], F16, kind="ExternalOutput").ap()

    xTr = xT.rearrange("(cc p) l -> p cc l", p=128)
    yTr = yT.rearrange("(cc p) l -> p cc l", p=128)
    outTr = outT.rearrange("(ct p) l -> p ct l", p=128)

    with tile.TileContext(nc) as tc:
        with (
            tc.tile_pool(name="prs", bufs=1) as prs,      # persistent SBUF
            tc.tile_pool(name="pt", bufs=4) as ptp,       # exp outputs
            tc.tile_pool(name="nrm", bufs=8) as nrm,      # 1/rowsum
            tc.tile_pool(name="stg", bufs=12) as stg,      # normalized O staging
            tc.tile_pool(name="ob", bufs=2) as obp,       # output staging
            tc.tile_pool(name="st", bufs=2, space="PSUM") as stp,   # scores/proj
            tc.tile_pool(name="ot", bufs=2, space="PSUM") as otp,   # AV accum
        ):
            # ---- persistent SBUF tensors
            x_sb = prs.tile([128, NCC, LQ], F16, tag="x")
            y_sb = prs.tile([128, NCTX, LKV], F16, tag="y")
            wq_sb = prs.tile([128, NCC, 256], F16, tag="wq")
            wk_sb = prs.tile([128, NCTX, 256], F16, tag="wk")
            wv_sb = prs.tile([128, NCTX, 256], F16, tag="wv")
            wp_sb = prs.tile([128, 2, C], F16, tag="wp")
            qt = prs.tile([128, 2, LQ], F16, tag="qt")      # [d(2h), pair, i]
            kt = prs.tile([128, 2, LKV], F16, tag="kt")     # [d(2h), pair, j]
            vaug = prs.tile([128, NJT, 4, 65], F16, tag="va")  # [j, jt, h, d|1]
            otn = prs.tile([128, 2, LQ], F16, tag="otn")    # [hd(2h), pair, i]
            ones = prs.tile([128, NJT, 4], F16, tag="ones")
            ident = prs.tile([128, 128], F16, tag="ident")
            scr = prs.tile([1, 2], F16, tag="scr")          # act-table warm

            # Act Exp table preload + identity for PE transposes (gpsimd)
            nc.vector.memset(scr[:], 0.0)
            nc.scalar.activation(scr[0:1, 0:1], scr[0:1, 1:2],
                                 mybir.ActivationFunctionType.Exp)
            make_identity(nc, ident[:])

            # ---- DMAs (SP queue, ordered to match consumption order)
            wqr = wq.rearrange("(cc p) h -> p cc h", p=128)
            nc.sync.dma_start(out=wq_sb[:, 0:4, :], in_=wqr[:, 0:4, :])
            nc.sync.dma_start(out=x_sb[:, 0:4, 0:512], in_=xTr[:, 0:4, 0:512])
            nc.sync.dma_start(out=wq_sb[:, 4:8, :], in_=wqr[:, 4:8, :])
            nc.sync.dma_start(out=x_sb[:, 4:8, 0:512], in_=xTr[:, 4:8, 0:512])
            nc.sync.dma_start(out=wk_sb, in_=wk.rearrange("(cc p) h -> p cc h", p=128))
            nc.sync.dma_start(out=y_sb[:, :, 0:256], in_=yTr[:, :, 0:256])
            nc.sync.dma_start(out=y_sb[:, :, 256:512], in_=yTr[:, :, 256:512])
            nc.sync.dma_start(out=wv_sb, in_=wv.rearrange("(cc p) h -> p cc h", p=128))
            for blk in range(1, 4):
                s = slice(blk * 512, (blk + 1) * 512)
                nc.sync.dma_start(out=x_sb[:, :, s], in_=xTr[:, :, s])
                nc.sync.dma_start(out=y_sb[:, :, s], in_=yTr[:, :, s])
            nc.sync.dma_start(out=wp_sb, in_=wp.rearrange("(r p) o -> p r o", p=128))

            nc.vector.memset(ones[:], 1.0)
            nc.vector.tensor_copy(
                vaug[:, :, :, 64:65],
                ones[:].rearrange("p j (h o) -> p j h o", o=1))

            # ---- unit emitters (independent chunks of PE work)
            def q_unit(pair, it):
                ps = stp.tile([128, 512], F32, tag="ps", name="ps_q")
                for cc in range(NCC):
                    nc.tensor.matmul(
                        ps[:], wq_sb[:, cc, pair * 128:(pair + 1) * 128],
                        x_sb[:, cc, it * 512:(it + 1) * 512],
                        start=(cc == 0), stop=(cc == NCC - 1))
                nc.vector.tensor_copy(qt[:, pair, it * 512:(it + 1) * 512], ps[:])

            def k_unit(pair, it, half=None):
                n = 512 if half is None else 256
                o = it * 512 + (0 if not half else (half - 1) * 256)
                ps = stp.tile([128, n], F32, tag="ps", name="ps_k")
                for cc in range(NCTX):
                    nc.tensor.matmul(
                        ps[:], wk_sb[:, cc, pair * 128:(pair + 1) * 128],
                        y_sb[:, cc, o:o + n],
                        start=(cc == 0), stop=(cc == NCTX - 1))
                nc.vector.tensor_copy(kt[:, pair, o:o + n], ps[:])

            def v_unit(pair, jt):
                # V proj for the 2 heads of `pair`, j-chunk jt
                ps = stp.tile([128, 128], F32, tag="ps", name="ps_v")
                for cc in range(NCTX):
                    nc.tensor.matmul(
                        ps[:], y_sb[:, cc, jt * 128:(jt + 1) * 128],
                        wv_sb[:, cc, pair * 128:(pair + 1) * 128],
                        start=(cc == 0), stop=(cc == NCTX - 1))
                nc.vector.tensor_copy(
                    vaug[:, jt, 2 * pair:2 * pair + 2, 0:64],
                    ps[:].rearrange("p (h d) -> p h d", d=64))

            def d_unit(it, ct, ob):
                # output projection for column-tile ct of i-block it. The
                # final i-block borrows the (dead by then) scores-pool tiles
                # to deepen the PSUM rotation, and splits its copies across
                # DVE and the idle Pool engine.
                if it == NIT - 1 and ct % 2 == 0:
                    ps = stp.tile([128, 512], F32, tag="st", name="ps_d2")
                else:
                    ps = stp.tile([128, 512], F32, tag="ps", name="ps_d")
                for pair in range(2):
                    nc.tensor.matmul(
                        ps[:], wp_sb[:, pair, ct * 128:(ct + 1) * 128],
                        otn[:, pair, it * 512:(it + 1) * 512],
                        start=(pair == 0), stop=(pair == 1))
                if it == NIT - 1 and ct % 2 == 0:
                    nc.scalar.copy(ob[:, ct, :], ps[:])
                else:
                    nc.vector.tensor_copy(ob[:, ct, :], ps[:])

            def t_unit(pair, it, onrms, pe=False):
                # transpose the 4 normalized [128,128] staging tiles of
                # (pair, it) into otn[:, pair, it-block]. Mid-stream blocks
                # use the DMA XBAR (free for PE/DVE; latency hidden by the
                # D-unit deadlines); the final block uses PE transposes to
                # keep the tail short.
                if not pe:
                    for isub in range(4):
                        base = it * 512 + isub * 128
                        nc.sync.dma_start(
                            out=otn[:, pair, base:base + 128],
                            in_=onrms[isub][:], transpose=True)
                    return
                tp = stp.tile([128, 4, 128], F16, tag="ps", name="ps_t")
                for isub in range(4):
                    for h in range(2):
                        nc.tensor.transpose(
                            tp[64 * h:64 * h + 64, isub, :],
                            onrms[isub][:, 64 * h:64 * h + 64],
                            ident[:])
                nc.vector.tensor_copy(
                    otn[:, pair, it * 512:(it + 1) * 512],
                    tp[:].rearrange("p a b -> p (a b)"))

            # ---- filler scheduling: units are (pe_rows, key, closure);
            # popped to keep cumulative filler rows at slot_idx *
            # FILL_ROWS_PER_SLOT, or on demand via ensure(key).
            filler = []
            pending = []   # [slots_left, [(rows, key, fn), ...]]
            emitted = set()
            state = {"slots": 0, "rows": 0}

            def push(units, delay=0):
                if delay:
                    pending.append([delay, list(units)])
                else:
                    filler.extend(units)

            def _pop_one():
                rows, key, fn = filler.pop(0)
                fn()
                state["rows"] += rows
                if key:
                    emitted.add(key)

            def ensure(key):
                """Force-emit queued filler up to (and incl.) `key`."""
                while key not in emitted and any(u[1] == key for u in filler):
                    _pop_one()

            def fill(flush=False):
                state["slots"] += 1
                for p in pending:
                    p[0] -= 1
                while pending and pending[0][0] <= 0:
                    filler.extend(pending.pop(0)[1])
                target = state["slots"] * FILL_ROWS_PER_SLOT
                while filler and (flush or state["rows"] < target):
                    _pop_one()

            def flush_all():
                while pending or filler:
                    fill(flush=True)

            # ---- attention pipeline pieces
            def scores_exp(pair, it, h, g0, glen):
                ensure(("q", pair, it))
                for blk in range(g0 // 4, (g0 + glen - 1) // 4 + 1):
                    ensure(("k", pair, blk))
                d0 = 64 * h
                st = stp.tile([128, glen, 512], F32, tag="st")
                for k in range(glen):
                    jt = g0 + k
                    nc.tensor.matmul(
                        st[:, k, :],
                        kt[d0:d0 + 64, pair, jt * 128:(jt + 1) * 128],
                        qt[d0:d0 + 64, pair, it * 512:(it + 1) * 512],
                        start=True, stop=True)
                pt = ptp.tile([128, glen, 512], F16, tag="pt")
                nc.scalar.activation(
                    pt[:], st[:], mybir.ActivationFunctionType.Exp, scale=SCALE)
                return pt

            ots = {}   # (pair, it) -> (ot_a, ot_b)
            obs = {}   # it -> output staging tile

            def av(pair, it, h, g0, glen, pt):
                for jt in range(g0, g0 + glen):
                    ensure(("v", pair, jt))
                if h == 0 and g0 == 0:
                    ot_a = otp.tile([128, 4, 128], F32, tag="ot", name="ot_a")
                    ot_b = otp.tile([128, 4, 128], F32, tag="ot", name="ot_b")
                    ots[(pair, it)] = (ot_a, ot_b)
                ot_h = ots[(pair, it)][h]
                for k in range(glen):
                    jt = g0 + k
                    for isub in range(4):
                        # one accumulation group per PSUM bank: start zeroes
                        # the whole 2KB zone lazily, so only the first write
                        # starts and only the last stops
                        nc.tensor.matmul(
                            ot_h[:, isub, 0:65],
                            pt[:, k, isub * 128:(isub + 1) * 128],
                            vaug[:, jt, 2 * pair + h, :],
                            start=(jt == 0 and isub == 0),
                            stop=(jt == NJT - 1 and isub == 3))

            def norm(pair, it):
                """1/rowsum + scale; returns the 4 staging tiles. For the
                final i-block, half the muls go on Act (idle by then)."""
                last = (pair == 1 and it == NIT - 1)
                ot_a, ot_b = ots.pop((pair, it))
                rs = []
                for h, ot_h in ((0, ot_a), (1, ot_b)):
                    rsinv = nrm.tile([128, 4], F32, tag="rs", name="rsinv")
                    nc.vector.reciprocal(rsinv[:], ot_h[:, :, 64:65])
                    rs.append(rsinv)
                last = (pair == 1 and it == NIT - 1)
                onrms = []
                for isub in range(4):
                    onrm = stg.tile([128, 128], F16, tag="onrm", name="onrm")
                    for h, ot_h in ((0, ot_a), (1, ot_b)):
                        if last and h == 1:
                            nc.scalar.mul(
                                onrm[:, 64 * h:64 * h + 64],
                                ot_h[:, isub, 0:64],
                                rs[h][:, isub:isub + 1])
                        else:
                            nc.vector.tensor_scalar_mul(
                                onrm[:, 64 * h:64 * h + 64],
                                ot_h[:, isub, 0:64],
                                rs[h][:, isub:isub + 1])
                    onrms.append(onrm)
                return onrms

            def on_it_boundary(pair, it):
                """Called right after the final AV of (pair, it) was emitted."""
                onrms = norm(pair, it)
                # transposes 2 slots later (lets the DVE muls drain first)
                push([(1024, None, lambda: t_unit(pair, it, onrms))], delay=2)
                if pair == 0 and it == 1:
                    push([(4096, ("q", 1, 0), lambda: q_unit(1, 0))])
                if pair == 0 and it == 2:
                    push([(4096, ("q", 1, 1), lambda: q_unit(1, 1))])
                if pair == 1:
                    if it + 2 < NIT:
                        push([(4096, ("q", 1, it + 2),
                               lambda: q_unit(1, it + 2))])
                    ob = obp.tile([128, 8, 512], F16, tag="ob", name="ob")
                    obs[it] = ob
                    # D(it) after the transpose unit (+DVE copy) lands; the
                    # stores ride the same FIFO so they are emitted after
                    # their D units (split small stores to shorten the tail)
                    units = []
                    for ct in range(8):
                        units.append(
                            (1024, None, lambda ct=ct: d_unit(it, ct, ob)))
                        if ct % 2 == 1:
                            units.append((0, None, lambda ct=ct: nc.sync.dma_start(
                                out=outTr[:, ct - 1:ct + 1,
                                          it * 512:(it + 1) * 512],
                                in_=ob[:, ct - 1:ct + 1, :])))
                    push(units, delay=5)

            # ---- prefix: minimal pair-0 prerequisites (PE ramps up here)
            q_unit(0, 0)
            k_unit(0, 0)
            for jt in range(4):
                v_unit(0, jt)

            # remaining prep becomes filler, ordered by first consumption
            push([(3072, ("k", 0, 1), lambda: k_unit(0, 1))])
            push([(768, ("v", 0, jt), lambda jt=jt: v_unit(0, jt))
                  for jt in range(4, 8)])
            push([(4096, ("q", 0, 1), lambda: q_unit(0, 1))])
            push([(3072, ("k", 0, 2), lambda: k_unit(0, 2))])
            push([(768, ("v", 0, jt), lambda jt=jt: v_unit(0, jt))
                  for jt in range(8, 12)])
            push([(4096, ("q", 0, 2), lambda: q_unit(0, 2))])
            push([(3072, ("k", 0, 3), lambda: k_unit(0, 3))])
            push([(768, ("v", 0, jt), lambda jt=jt: v_unit(0, jt))
                  for jt in range(12, 16)])
            push([(4096, ("q", 0, 3), lambda: q_unit(0, 3))])
            push([(3072, ("k", 1, it), lambda it=it: k_unit(1, it))
                  for it in range(4)])
            push([(768, ("v", 1, jt), lambda jt=jt: v_unit(1, jt))
                  for jt in range(16)])
            emitted.update({("q", 0, 0), ("k", 0, 0)})

            # ---- main pipeline: scores/exp run 2 slots ahead of AV
            items = [(0, 0, h, g0, glen) for (g0, glen) in GROUPS
                     for h in range(2)]
            items += [(pair, it, h, g0, glen)
                      for pair in range(2) for it in range(NIT)
                      for h in range(2) for (g0, glen) in GROUPS
                      if not (pair == 0 and it == 0)]
            inflight = []

            def retire(slot_item):
                pair, it, h, g0, glen, pt = slot_item
                av(pair, it, h, g0, glen, pt)
                if h == 1 and g0 + glen == NJT:
                    on_it_boundary(pair, it)

            for item in items:
                pair, it, h, g0, glen = item
                pt = scores_exp(pair, it, h, g0, glen)
                inflight.append((pair, it, h, g0, glen, pt))
                if len(inflight) > 7:
                    retire(inflight.pop(0))
                fill()
            while inflight:
                retire(inflight.pop(0))
                fill()
            flush_all()


    nc.compile()
    return nc


_NC_CACHE = {}


def _get_nc():
    if "nc" not in _NC_CACHE:
        _NC_CACHE["nc"] = build_kernel()
    return _NC_CACHE["nc"]


def make_in_maps(x, y, Wq, Wkv, Wproj):
    """Host-side sharding: core = b * 4 + hg (hg = 4-head group)."""
    x = np.asarray(x, dtype=np.float32)
    y = np.asarray(y, dtype=np.float32)
    Wq = np.asarray(Wq, dtype=np.float32)
    Wkv = np.asarray(Wkv, dtype=np.float32).reshape(CTX, 2, H, DK)
    Wproj = np.asarray(Wproj, dtype=np.float32)

    in_maps = []
    for core in range(8):
        b, hg = core // 4, core % 4
        hs = slice(4 * hg, 4 * hg + 4)
        in_maps.append({
            "xT": np.ascontiguousarray(x[b].T).astype(np.float16),
            "yT": np.ascontiguousarray(y[b].T).astype(np.float16),
            "wq": np.ascontiguousarray(
                Wq[:, 4 * hg * DK:(4 * hg + 4) * DK]).astype(np.float16),
            "wk": np.ascontiguousarray(
                Wkv[:, 0, hs, :].reshape(CTX, 4 * DK)).astype(np.float16),
            "wv": np.ascontiguousarray(
                Wkv[:, 1, hs, :].reshape(CTX, 4 * DK)).astype(np.float16),
            "wp": np.ascontiguousarray(
                Wproj[4 * hg * DK:(4 * hg + 4) * DK, :]).astype(np.float16),
        })
    return in_maps


def kernel(x, y, Wq, Wkv, Wproj, bproj):
    nc = _get_nc()
    in_maps = make_in_maps(x, y, Wq, Wkv, Wproj)
    res = run_bass_kernel_spmd(nc, in_maps, core_ids=list(range(8)))
    bproj = np.asarray(bproj, dtype=np.float32)
    out = np.empty((B, LQ, C), dtype=np.float32)
    for b in range(B):
        acc = res.results[4 * b]["outT"].astype(np.float32).copy()
        for hg in range(1, 4):
            acc += res.results[4 * b + hg]["outT"]
        out[b] = acc.T + bproj
    return out
